# revision 26
# baseline (speedup 1.0000x reference)
"""Trainium2 Bass kernel for nn_ATTMILLoss.

Reference computation:
    rows[b,n,:]  = syb_graph[b, idx_of_objs[b,n], :]            (gather)
    pos[k,b,n]   = sum_l att[k,b,n,l] * (rows[b,n,l] > 0)
    neg[k,b,n]   = sum_l att[k,b,n,l] * (rows[b,n,l] == 0)
    loss         = mean(relu(MARGIN - (pos - neg)))

Since rows in {0,1}: pos - neg = sum_l att[k,b,n,l] * (2*rows[b,n,l] - 1),
and since att >= 0, att*(+-1) is just an IEEE sign-bit flip.

Strategy (8 cores, data-parallel over batch):
  Each core gets 16 batches. The gather is pure index shuffling, so the
  host performs it while sharding, and ships:
    - att as fp8 e4m3 (quantization gives ~6e-4 rel error on the final
      loss vs the 2e-2 gate), host-transposed so the l (summation) axis
      sits on SBUF partitions in contiguous 1.5 MiB slabs
      (12 KiB/partition runs), with l-chunk PAIRS interleaved for the
      PE DoubleRow mode: slab (bg, lcp) holds l-chunks {2*lcp, 2*lcp+1}
      as [p, ko, 4 batches, 6 blocks, n];
    - the sign mask as uint16 over adjacent-n fp8 pairs, resident.
  Device: DVE flips sign bits with tensor_tensor bitwise_xor on the
  uint16 view (builtin TT op -> 2x perf mode; XOR is grouping-agnostic
  so fp8 pairs ride the 16-bit path), two ops per slab (one per ko).
  The PE reduces over l with perf_mode=DoubleRow fp8 matmuls against a
  ones vector: contraction 256 (both ko chunks at once), so each (b,k)
  diff row takes 2 matmuls instead of 4, accumulating fp32 into a
  [1,512] PSUM bank. ACT drains each bank with relu(margin-x)+accum;
  host sums 8x96 partials.

  v1  (indirect gathers, f32, fused DVE): 351 us.
  v4  (host signs int8, bf16, fused DVE): 284 us, DVE-bound.
  v5  (bf16 + XOR + PE reduce): 201 us.
  v6  (fp8 pairs): 161 us, phase-serialized.
  v7  (mask up-front, 3-group lookahead): 138 us, PE-paced.
"""

import sys

for _p in ("/opt/trn_rl_repo",):
    if _p not in sys.path:
        sys.path.insert(0, _p)

import numpy as np

BLOCKS, BATCH, N, L = 6, 128, 512, 512
MARGIN = 0.6
NCORES = 8
BPC = BATCH // NCORES  # batches per core
P = 128
KO = 2  # l-chunks per DoubleRow matmul
LCP = L // P // KO  # 2 chunk-pairs; l = (lcp*KO + ko)*P + p
BG = 4  # batches per slab
NBG = BPC // BG
N2 = N // 2  # fp8 pairs per row
NROWS = BPC * BLOCKS  # 96 loss partials, one per (b, k)

_CACHE = {}


def _build_program():
    import concourse.bacc as bacc
    import concourse.bass as bass
    import concourse.mybir as mybir
    import concourse.tile as tile

    nc = bacc.Bacc("TRN2", target_bir_lowering=False, debug=False)

    # att: contiguous 1.5 MiB fp8 slabs, one per (bg, lcp); inside a
    # slab partition p=l owns [KO, BG, BLOCKS, N] fp8 (12 KiB).
    att = nc.dram_tensor(
        "att",
        [NBG, LCP, P, KO, BG, BLOCKS, N],
        mybir.dt.uint8,
        kind="ExternalInput",
    )
    # mask: per-fp8-pair sign bits; trailing 1-dim is the broadcast
    # slot for the BLOCKS axis.
    mask = nc.dram_tensor(
        "mask", [P, NBG, LCP, KO, BG, 1, N2], mybir.dt.uint16, kind="ExternalInput"
    )
    out = nc.dram_tensor("out", [1, NROWS], mybir.dt.float32, kind="ExternalOutput")

    with tile.TileContext(nc) as tc:
        with (
            tc.tile_pool(name="constp", bufs=1) as constp,
            tc.tile_pool(name="attp", bufs=6) as attp,
            tc.psum_pool(name="psump", bufs=8) as psump,
            tc.tile_pool(name="outp", bufs=2) as outp,
        ):
            margin_t = constp.tile([P, 1], mybir.dt.float32)
            nc.gpsimd.memset(margin_t[:], MARGIN)
            # DoubleRow stationary: [Ki, Ko, dim] with 16B ko stride.
            ones_t = constp.tile([P, KO, 16], mybir.dt.float8e4)
            nc.gpsimd.memset(ones_t[:], 1.0)

            mask_t = constp.tile([P, NBG, LCP, KO, BG, 1, N2], mybir.dt.uint16)
            partial = constp.tile([1, NROWS], mybir.dt.float32)

            # All mask slices up front on the otherwise-idle sync ring
            # so the first XOR's mask dependency lands within ~5 us.
            for bg in range(NBG):
                nc.sync.dma_start(
                    out=mask_t[:, bg], in_=mask[:, bg]
                )

            rings = [nc.scalar, nc.gpsimd]
            di = 0
            for bg in range(NBG):
                att_tiles = {}
                for lcp in range(LCP):
                    att_t = attp.tile(
                        [P, KO, BG, BLOCKS, N], mybir.dt.uint8, tag="att"
                    )
                    att_tiles[lcp] = att_t
                    rings[di % len(rings)].dma_start(
                        out=att_t[:], in_=att[bg, lcp]
                    )
                    di += 1
                    # In-place sign flip on the uint16 pair view: one
                    # 2x-mode DVE op per (slab, ko).
                    for ko in range(KO):
                        v16 = att_t[:, ko].bitcast(mybir.dt.uint16)
                        nc.vector.tensor_tensor(
                            out=v16,
                            in0=v16,
                            in1=mask_t[:, bg, lcp, ko].broadcast_to(
                                [P, BG, BLOCKS, N2]
                            ),
                            op=mybir.AluOpType.bitwise_xor,
                        )
                # PE reduce over l: DoubleRow ones.T @ signed-att
                # contracts 256 rows (both ko chunks) per matmul; two
                # matmuls (lcp pair) accumulate diff[b,k,:] in fp32 in
                # a [1,N] PSUM tile. ACT drains each with
                # relu(margin-x)+accum -> partial[0, b*BLOCKS+k].
                for b2 in range(BG):
                    for k in range(BLOCKS):
                        q = (bg * BG + b2) * BLOCKS + k
                        psum_t = psump.tile([1, N], mybir.dt.float32)
                        for lcp in range(LCP):
                            nc.tensor.matmul(
                                psum_t[:],
                                lhsT=ones_t[:, :, 0:1],
                                rhs=att_tiles[lcp][:, :, b2, k, :].bitcast(
                                    mybir.dt.float8e4
                                ),
                                start=(lcp == 0),
                                stop=(lcp == LCP - 1),
                                perf_mode=mybir.MatmulPerfMode.DoubleRow,
                            )
                        relu_t = outp.tile([1, N], mybir.dt.float32)
                        nc.scalar.activation(
                            out=relu_t[:],
                            in_=psum_t[:],
                            func=mybir.ActivationFunctionType.Relu,
                            scale=-1.0,
                            bias=margin_t[:1],
                            accum_out=partial[:, q : q + 1],
                        )

            nc.sync.dma_start(out=out[:], in_=partial[:])

    nc.compile()
    return nc


def _get_program():
    if "nc" not in _CACHE:
        _CACHE["nc"] = _build_program()
    return _CACHE["nc"]


def _shard_inputs(idx_of_objs, syb_graph, att_weights):
    # Host performs the row gather (index shuffling only) and the
    # layout/dtype transforms; all arithmetic stays on device.
    import ml_dtypes

    rows = np.take_along_axis(
        syb_graph, idx_of_objs[:, :, None].astype(np.int64), axis=1
    )  # [BATCH, N, L] in {0,1}
    # sign-bit byte where the row is 0 (negative weight)
    m8 = ((rows == 0).astype(np.uint8)) << 7
    # [BATCH, N, L] -> [core, P, NBG, LCP, KO, BG, N]; l=(lcp*KO+ko)*P+p
    m8 = np.ascontiguousarray(
        m8.reshape(NCORES, NBG, BG, N, LCP, KO, P).transpose(0, 6, 1, 4, 5, 2, 3)
    )
    m16 = m8.view(np.uint16).reshape(NCORES, P, NBG, LCP, KO, BG, 1, N2)
    # att: f32 -> fp8 e4m3 bytes -> [core, NBG, LCP, P, KO, BG, BLOCKS, N]
    att8 = att_weights.astype(ml_dtypes.float8_e4m3).view(np.uint8)
    att8 = np.ascontiguousarray(
        att8.reshape(BLOCKS, NCORES, NBG, BG, N, LCP, KO, P).transpose(
            1, 2, 5, 7, 6, 3, 0, 4
        )
    )
    return [{"att": att8[c], "mask": m16[c]} for c in range(NCORES)]


def kernel(idx_of_objs, valid2all, syb_graph, att_weights, vis_len):
    from concourse.bass_utils import run_bass_kernel_spmd

    del valid2all, vis_len  # no-ops given the reference's setup
    idx_of_objs = np.asarray(idx_of_objs, dtype=np.int32)
    syb_graph = np.asarray(syb_graph, dtype=np.int32)
    att_weights = np.asarray(att_weights, dtype=np.float32)

    nc = _get_program()
    in_maps = _shard_inputs(idx_of_objs, syb_graph, att_weights)
    res = run_bass_kernel_spmd(nc, in_maps, list(range(NCORES)))
    total = 0.0
    for r in res.results:
        total += float(np.asarray(r["out"], dtype=np.float64).sum())
    loss = total / (BLOCKS * BATCH * N)
    return np.float32(loss)


if __name__ == "__main__":
    _build_program()
    print("BUILD OK")
